# revision 8
# baseline (speedup 1.0000x reference)
"""Causal self-attention (single head) on 8 TRN2 NeuronCores.

Problem: x [4, 2048, 1024] f32; W_q/W_k [1024, 256]; W_v [1024, 1024].
Returns (attn_output [4,2048,1024], attn_weights [4,2048,2048]).

Sharding: 8 cores = 4 batches x 2-way tensor-parallel over W_v/output
columns.  Each core computes the full causal softmax for its batch
(needed for its half of A@V); core h=0 of each pair writes the full
attn_weights.

Per-core kernel (all matmuls in float32r: ~1e-4 rel err, bf16-speed):
  xT       = PE-transpose(x)                  [1024, 2048]
  QT/KT    = Wq/Wk^T @ xT  (f32r)             [256, 2048]
  V        = xT^T @ Wv_half                   [2048, 512]
  ST[k,q]  = KT^T-chunks @ QT   (scoresT, PSUM f32)
  PT       = exp(ST/16) masked (causal)       -> A@V lhsT
  S[q,k]   = QT^T-chunks @ KT   (scores)
  ES       = exp(S/16) masked; rowsum via accum_out; A rows = ES * (1/sum)
  out rows = (sum_k PT_k^T @ V_k) * (1/sum)
"""
import os
import sys
import numpy as np

sys.path.insert(0, "/opt/trn_rl_repo")

B, T, D, DA, E = 4, 2048, 1024, 256, 512
NT = T // 128          # 16 t-blocks
ND = D // 128          # 8 d-chunks
G = T // 512           # 4 q-groups of 512
SCALE = 1.0 / 16.0     # 1/sqrt(d_attn)

_cache = {}


def _build():
    import concourse.bacc as bacc
    import concourse.tile as tile
    from concourse import mybir

    f32 = mybir.dt.float32
    f32r = mybir.dt.float32r
    Exp = mybir.ActivationFunctionType.Exp
    is_ge = mybir.AluOpType.is_ge
    X = mybir.AxisListType.X

    nc = bacc.Bacc()
    x_d = nc.declare_dram_parameter("x", [T, D], f32r, isOutput=False)
    wq_d = nc.declare_dram_parameter("wq", [D, DA], f32r, isOutput=False)
    wk_d = nc.declare_dram_parameter("wk", [D, DA], f32r, isOutput=False)
    wv_d = nc.declare_dram_parameter("wv", [D, E], f32r, isOutput=False)
    aw_d = nc.declare_dram_parameter("aw", [T, T], f32, isOutput=True)
    ao_d = nc.declare_dram_parameter("ao", [T, E], f32, isOutput=True)

    with tile.TileContext(nc) as tc:
        with (
            tc.tile_pool(name="persist", bufs=1) as persist,
            tc.tile_pool(name="xload", bufs=3) as xload,
            tc.tile_pool(name="xtp", bufs=2) as xtp,
            tc.tile_pool(name="ptp", bufs=18) as ptp,
            tc.tile_pool(name="esp", bufs=2) as esp,
            tc.tile_pool(name="small", bufs=16) as small,
            tc.tile_pool(name="ostp", bufs=3) as ostp,
            tc.tile_pool(name="ps", bufs=8, space="PSUM") as ps,
        ):
            # ---- persistent tiles ----
            ident_f32 = persist.tile([128, 128], f32)
            nc.gpsimd.memset(ident_f32[:], 0.0)
            nc.gpsimd.affine_select(
                out=ident_f32[:], in_=ident_f32[:],
                compare_op=mybir.AluOpType.not_equal, fill=1.0,
                base=0, pattern=[[-1, 128]], channel_multiplier=1,
            )
            ident = persist.tile([128, 128], f32r)
            nc.vector.tensor_copy(ident[:], ident_f32[:])
            # weight loads go on the gpsimd (SWDGE) queue so they don't
            # delay the x-tile loads feeding the first PE transposes
            wq = persist.tile([128, ND, DA], f32r)
            wk = persist.tile([128, ND, DA], f32r)
            wv = persist.tile([128, ND, E], f32r)
            nc.gpsimd.dma_start(wq[:], wq_d.ap().rearrange("(c p) a -> p c a", p=128))
            nc.gpsimd.dma_start(wk[:], wk_d.ap().rearrange("(c p) a -> p c a", p=128))
            nc.gpsimd.dma_start(wv[:], wv_d.ap().rearrange("(c p) e -> p c e", p=128))
            qt = persist.tile([128, 2, T], f32r)   # [a%128, a//128, t]
            kt = persist.tile([128, 2, T], f32r)
            v = persist.tile([128, NT, E], f32r)   # [k%128, k//128, e]

            # ---- phase A: transpose + projections, per quad of t-blocks ----
            for p in range(NT // 4):
                xt_quad = xtp.tile([128, ND, 512], f32r)  # xT columns, 4 blocks
                for half in range(4):
                    ti = 4 * p + half
                    xt_in = xload.tile([128, D], f32r, tag="xin")
                    nc.sync.dma_start(xt_in[:], x_d.ap()[ti * 128:(ti + 1) * 128, :])
                    for j in range(ND):
                        tp_ps = ps.tile([128, 512], f32r, tag="ps")
                        nc.tensor.matmul(
                            tp_ps[:, :128], xt_in[:, j * 128:(j + 1) * 128],
                            ident[:], is_transpose=True,
                        )
                        nc.vector.tensor_copy(
                            xt_quad[:, j, half * 128:(half + 1) * 128],
                            tp_ps[:, :128],
                        )
                # QT / KT partial columns  [128a, 512t]
                for c in range(2):
                    for w_sb, o_sb in ((wq, qt), (wk, kt)):
                        pr_ps = ps.tile([128, 512], f32, tag="ps")
                        for j in range(ND):
                            nc.tensor.matmul(
                                pr_ps[:],
                                w_sb[:, j, c * 128:(c + 1) * 128],
                                xt_quad[:, j, :],
                                start=(j == 0), stop=(j == ND - 1),
                            )
                        nc.vector.tensor_copy(
                            o_sb[:, c, p * 512:(p + 1) * 512], pr_ps[:]
                        )
                # V rows for the four t-blocks  [128t, 512e]
                for half in range(4):
                    ti = 4 * p + half
                    v_ps = ps.tile([128, 512], f32, tag="ps")
                    for j in range(ND):
                        nc.tensor.matmul(
                            v_ps[:],
                            xt_quad[:, j, half * 128:(half + 1) * 128],
                            wv[:, j, :],
                            start=(j == 0), stop=(j == ND - 1),
                        )
                    nc.vector.tensor_copy(v[:, ti, :], v_ps[:])

            # ---- phase B: attention, per q-group of 512 ----
            for g in range(G):
                nk = 4 * g + 4          # k-blocks participating in this group
                pt_tiles = []
                for k in range(nk):
                    st_ps = ps.tile([128, 512], f32, tag="ps")
                    for c in range(2):
                        nc.tensor.matmul(
                            st_ps[:],
                            kt[:, c, k * 128:(k + 1) * 128],
                            qt[:, c, g * 512:(g + 1) * 512],
                            start=(c == 0), stop=(c == 1),
                        )
                    pt_k = ptp.tile([128, 512], f32r, tag="pt")
                    nc.scalar.activation(pt_k[:], st_ps[:], Exp, scale=SCALE)
                    if k >= 4 * g:
                        # keep where k_global <= q_global:
                        # -k_local + q_local + (512g - 128k) >= 0
                        nc.gpsimd.affine_select(
                            out=pt_k[:], in_=pt_k[:], compare_op=is_ge,
                            fill=0.0, base=512 * g - 128 * k,
                            pattern=[[1, 512]], channel_multiplier=-1,
                        )
                    pt_tiles.append(pt_k)

                for ii in range(4):
                    i = 4 * g + ii
                    es_i = esp.tile([128, T], f32, tag="es")
                    tot = None
                    for kg in range(g + 1):
                        s_ps = ps.tile([128, 512], f32, tag="ps")
                        for c in range(2):
                            nc.tensor.matmul(
                                s_ps[:],
                                qt[:, c, i * 128:(i + 1) * 128],
                                kt[:, c, kg * 512:(kg + 1) * 512],
                                start=(c == 0), stop=(c == 1),
                            )
                        pa = small.tile([128, 1], f32, tag="pa")
                        sl = es_i[:, kg * 512:(kg + 1) * 512]
                        if kg < g:
                            nc.scalar.activation(sl, s_ps[:], Exp, scale=SCALE,
                                                 accum_out=pa[:])
                        else:
                            nc.scalar.activation(sl, s_ps[:], Exp, scale=SCALE)
                            # keep where 512g + k_local <= 128i + q_local:
                            # q_local - k_local + 128*ii >= 0
                            nc.gpsimd.affine_select(
                                out=sl, in_=sl, compare_op=is_ge,
                                fill=0.0, base=128 * ii,
                                pattern=[[-1, 512]], channel_multiplier=1,
                            )
                            nc.vector.reduce_sum(pa[:], sl, axis=X)
                        if tot is None:
                            tot = pa
                        else:
                            nc.vector.tensor_add(tot[:], tot[:], pa[:])
                    rr = small.tile([128, 1], f32, tag="rr")
                    nc.vector.reciprocal(rr[:], tot[:])
                    # normalized attention rows -> DRAM, sliced to pipeline
                    for kg in range(g + 1):
                        sl = es_i[:, kg * 512:(kg + 1) * 512]
                        nc.vector.tensor_scalar_mul(sl, in0=sl, scalar1=rr[:])
                        nc.sync.dma_start(
                            aw_d.ap()[i * 128:(i + 1) * 128,
                                      kg * 512:(kg + 1) * 512],
                            sl,
                        )
                    # A @ V for this q-block
                    av_ps = ps.tile([128, 512], f32, tag="ps")
                    for k in range(i + 1):
                        nc.tensor.matmul(
                            av_ps[:],
                            pt_tiles[k][:, ii * 128:(ii + 1) * 128],
                            v[:, k, :],
                            start=(k == 0), stop=(k == i),
                        )
                    ob = ostp.tile([128, 512], f32, tag="ob")
                    nc.vector.tensor_scalar_mul(ob[:], in0=av_ps[:], scalar1=rr[:])
                    nc.sync.dma_start(ao_d.ap()[i * 128:(i + 1) * 128, :], ob[:])

    nc.compile()
    return nc


def _get_nc():
    if "nc" not in _cache:
        _cache["nc"] = _build()
    return _cache["nc"]


def kernel(x, W_q, W_k, W_v):
    from concourse.bass_utils import run_bass_kernel_spmd

    x = np.ascontiguousarray(np.asarray(x, dtype=np.float32))
    W_q = np.ascontiguousarray(np.asarray(W_q, dtype=np.float32))
    W_k = np.ascontiguousarray(np.asarray(W_k, dtype=np.float32))
    W_v = np.ascontiguousarray(np.asarray(W_v, dtype=np.float32))

    nc = _get_nc()
    in_maps = []
    for core in range(8):
        b, h = core // 2, core % 2
        in_maps.append({
            "x": x[b],
            "wq": W_q,
            "wk": W_k,
            "wv": np.ascontiguousarray(W_v[:, h * E:(h + 1) * E]),
        })
    res = run_bass_kernel_spmd(
        nc, in_maps, core_ids=list(range(8)),
        trace=bool(int(os.environ.get("KERNEL_TRACE", "0"))),
    )
    _cache["last_results"] = res
    attn_weights = np.stack([res.results[2 * b]["aw"] for b in range(B)])
    attn_output = np.stack([
        np.concatenate([res.results[2 * b]["ao"], res.results[2 * b + 1]["ao"]],
                       axis=1)
        for b in range(B)
    ])
    return attn_output, attn_weights


# revision 9
# speedup vs baseline: 1.0513x; 1.0513x over previous
"""Causal self-attention (single head) on 8 TRN2 NeuronCores.

Problem: x [4, 2048, 1024] f32; W_q/W_k [1024, 256]; W_v [1024, 1024].
Returns (attn_output [4,2048,1024], attn_weights [4,2048,2048]).

Sharding: 8 cores = 4 batches x 2-way tensor-parallel over W_v/output
columns.  Each core computes the full causal softmax for its batch
(needed for its half of A@V); core h=0 of each pair writes the full
attn_weights.

Per-core kernel (all matmuls in float32r: ~1e-4 rel err, bf16-speed):
  xT       = PE-transpose(x)                  [1024, 2048]
  QT/KT    = Wq/Wk^T @ xT  (f32r)             [256, 2048]
  V        = xT^T @ Wv_half                   [2048, 512]
  ST[k,q]  = KT^T-chunks @ QT   (scoresT, PSUM f32)
  PT       = exp(ST/16) masked (causal)       -> A@V lhsT
  S[q,k]   = QT^T-chunks @ KT   (scores)
  ES       = exp(S/16) masked; rowsum via accum_out; A rows = ES * (1/sum)
  out rows = (sum_k PT_k^T @ V_k) * (1/sum)
"""
import os
import sys
import numpy as np

sys.path.insert(0, "/opt/trn_rl_repo")

B, T, D, DA, E = 4, 2048, 1024, 256, 512
NT = T // 128          # 16 t-blocks
ND = D // 128          # 8 d-chunks
G = T // 512           # 4 q-groups of 512
SCALE = 1.0 / 16.0     # 1/sqrt(d_attn)

_cache = {}


def _build():
    import concourse.bacc as bacc
    import concourse.tile as tile
    from concourse import mybir

    f32 = mybir.dt.float32
    f32r = mybir.dt.float32r
    Exp = mybir.ActivationFunctionType.Exp
    is_ge = mybir.AluOpType.is_ge
    X = mybir.AxisListType.X

    nc = bacc.Bacc()
    x_d = nc.declare_dram_parameter("x", [T, D], f32r, isOutput=False)
    wq_d = nc.declare_dram_parameter("wq", [D, DA], f32r, isOutput=False)
    wk_d = nc.declare_dram_parameter("wk", [D, DA], f32r, isOutput=False)
    wv_d = nc.declare_dram_parameter("wv", [D, E], f32r, isOutput=False)
    aw_d = nc.declare_dram_parameter("aw", [T, T], f32, isOutput=True)
    ao_d = nc.declare_dram_parameter("ao", [T, E], f32, isOutput=True)

    with tile.TileContext(nc) as tc:
        with (
            tc.tile_pool(name="persist", bufs=1) as persist,
            tc.tile_pool(name="xload", bufs=3) as xload,
            tc.tile_pool(name="xtp", bufs=3) as xtp,
            tc.tile_pool(name="ptp", bufs=20) as ptp,
            tc.tile_pool(name="esp", bufs=2) as esp,
            tc.tile_pool(name="small", bufs=12) as small,
            tc.tile_pool(name="ostp", bufs=2) as ostp,
            tc.tile_pool(name="ps", bufs=8, space="PSUM") as ps,
        ):
            # ---- persistent tiles ----
            ident_f32 = persist.tile([128, 128], f32)
            nc.gpsimd.memset(ident_f32[:], 0.0)
            nc.gpsimd.affine_select(
                out=ident_f32[:], in_=ident_f32[:],
                compare_op=mybir.AluOpType.not_equal, fill=1.0,
                base=0, pattern=[[-1, 128]], channel_multiplier=1,
            )
            ident = persist.tile([128, 128], f32r)
            nc.vector.tensor_copy(ident[:], ident_f32[:])
            # weight loads go on the gpsimd (SWDGE) queue so they don't
            # delay the x-tile loads feeding the first PE transposes
            wq = persist.tile([128, ND, DA], f32r)
            wk = persist.tile([128, ND, DA], f32r)
            wv = persist.tile([128, ND, E], f32r)
            nc.gpsimd.dma_start(wq[:], wq_d.ap().rearrange("(c p) a -> p c a", p=128))
            nc.gpsimd.dma_start(wk[:], wk_d.ap().rearrange("(c p) a -> p c a", p=128))
            nc.gpsimd.dma_start(wv[:], wv_d.ap().rearrange("(c p) e -> p c e", p=128))
            qt = persist.tile([128, 2, T], f32r)   # [a%128, a//128, t]
            kt = persist.tile([128, 2, T], f32r)
            v = persist.tile([128, NT, E], f32r)   # [k%128, k//128, e]

            # ---- phase A: transpose + projections, per pair of t-blocks ----
            for p in range(NT // 2):
                xt_pair = xtp.tile([128, ND, 256], f32r)  # xT columns, 2 blocks
                for half in range(2):
                    ti = 2 * p + half
                    xt_in = xload.tile([128, D], f32r, tag="xin")
                    nc.sync.dma_start(xt_in[:], x_d.ap()[ti * 128:(ti + 1) * 128, :])
                    for j in range(ND):
                        tp_ps = ps.tile([128, 512], f32r, tag="ps")
                        nc.tensor.matmul(
                            tp_ps[:, :128], xt_in[:, j * 128:(j + 1) * 128],
                            ident[:], is_transpose=True,
                        )
                        # alternate copy-out engine so PSUM drain keeps up
                        # with the PE transpose stream
                        eng = nc.vector if (j % 2 == 0) else nc.scalar
                        if eng is nc.vector:
                            eng.tensor_copy(
                                xt_pair[:, j, half * 128:(half + 1) * 128],
                                tp_ps[:, :128],
                            )
                        else:
                            nc.scalar.copy(
                                xt_pair[:, j, half * 128:(half + 1) * 128],
                                tp_ps[:, :128],
                            )
                # QT / KT partial columns  [128a, 256t]
                for c in range(2):
                    for w_sb, o_sb in ((wq, qt), (wk, kt)):
                        pr_ps = ps.tile([128, 512], f32, tag="ps")
                        for j in range(ND):
                            nc.tensor.matmul(
                                pr_ps[:, :256],
                                w_sb[:, j, c * 128:(c + 1) * 128],
                                xt_pair[:, j, :],
                                start=(j == 0), stop=(j == ND - 1),
                            )
                        nc.vector.tensor_copy(
                            o_sb[:, c, p * 256:(p + 1) * 256], pr_ps[:, :256]
                        )
                # V rows for the two t-blocks  [128t, 512e]
                for half in range(2):
                    ti = 2 * p + half
                    v_ps = ps.tile([128, 512], f32, tag="ps")
                    for j in range(ND):
                        nc.tensor.matmul(
                            v_ps[:],
                            xt_pair[:, j, half * 128:(half + 1) * 128],
                            wv[:, j, :],
                            start=(j == 0), stop=(j == ND - 1),
                        )
                    nc.vector.tensor_copy(v[:, ti, :], v_ps[:])

            # ---- phase B: attention, per q-group of 512 ----
            for g in range(G):
                nk = 4 * g + 4          # k-blocks participating in this group
                pt_tiles = []
                for k in range(nk):
                    st_ps = ps.tile([128, 512], f32, tag="ps")
                    for c in range(2):
                        nc.tensor.matmul(
                            st_ps[:],
                            kt[:, c, k * 128:(k + 1) * 128],
                            qt[:, c, g * 512:(g + 1) * 512],
                            start=(c == 0), stop=(c == 1),
                        )
                    pt_k = ptp.tile([128, 512], f32r, tag="pt")
                    nc.scalar.activation(pt_k[:], st_ps[:], Exp, scale=SCALE)
                    if k >= 4 * g:
                        # keep where k_global <= q_global:
                        # -k_local + q_local + (512g - 128k) >= 0
                        nc.gpsimd.affine_select(
                            out=pt_k[:], in_=pt_k[:], compare_op=is_ge,
                            fill=0.0, base=512 * g - 128 * k,
                            pattern=[[1, 512]], channel_multiplier=-1,
                        )
                    pt_tiles.append(pt_k)

                for ii in range(4):
                    i = 4 * g + ii
                    es_i = esp.tile([128, T], f32, tag="es")
                    tot = None
                    for kg in range(g + 1):
                        s_ps = ps.tile([128, 512], f32, tag="ps")
                        for c in range(2):
                            nc.tensor.matmul(
                                s_ps[:],
                                qt[:, c, i * 128:(i + 1) * 128],
                                kt[:, c, kg * 512:(kg + 1) * 512],
                                start=(c == 0), stop=(c == 1),
                            )
                        pa = small.tile([128, 1], f32, tag="pa")
                        sl = es_i[:, kg * 512:(kg + 1) * 512]
                        if kg < g:
                            nc.scalar.activation(sl, s_ps[:], Exp, scale=SCALE,
                                                 accum_out=pa[:])
                        else:
                            nc.scalar.activation(sl, s_ps[:], Exp, scale=SCALE)
                            # keep where 512g + k_local <= 128i + q_local:
                            # q_local - k_local + 128*ii >= 0
                            nc.gpsimd.affine_select(
                                out=sl, in_=sl, compare_op=is_ge,
                                fill=0.0, base=128 * ii,
                                pattern=[[-1, 512]], channel_multiplier=1,
                            )
                            nc.vector.reduce_sum(pa[:], sl, axis=X)
                        if tot is None:
                            tot = pa
                        else:
                            nc.vector.tensor_add(tot[:], tot[:], pa[:])
                    rr = small.tile([128, 1], f32, tag="rr")
                    nc.vector.reciprocal(rr[:], tot[:])
                    # normalized attention rows -> DRAM, sliced to pipeline
                    for kg in range(g + 1):
                        sl = es_i[:, kg * 512:(kg + 1) * 512]
                        nc.vector.tensor_scalar_mul(sl, in0=sl, scalar1=rr[:])
                        nc.sync.dma_start(
                            aw_d.ap()[i * 128:(i + 1) * 128,
                                      kg * 512:(kg + 1) * 512],
                            sl,
                        )
                    # A @ V for this q-block
                    av_ps = ps.tile([128, 512], f32, tag="ps")
                    for k in range(i + 1):
                        nc.tensor.matmul(
                            av_ps[:],
                            pt_tiles[k][:, ii * 128:(ii + 1) * 128],
                            v[:, k, :],
                            start=(k == 0), stop=(k == i),
                        )
                    ob = ostp.tile([128, 512], f32, tag="ob")
                    nc.vector.tensor_scalar_mul(ob[:], in0=av_ps[:], scalar1=rr[:])
                    nc.sync.dma_start(ao_d.ap()[i * 128:(i + 1) * 128, :], ob[:])

    nc.compile()
    return nc


def _get_nc():
    if "nc" not in _cache:
        _cache["nc"] = _build()
    return _cache["nc"]


def kernel(x, W_q, W_k, W_v):
    from concourse.bass_utils import run_bass_kernel_spmd

    x = np.ascontiguousarray(np.asarray(x, dtype=np.float32))
    W_q = np.ascontiguousarray(np.asarray(W_q, dtype=np.float32))
    W_k = np.ascontiguousarray(np.asarray(W_k, dtype=np.float32))
    W_v = np.ascontiguousarray(np.asarray(W_v, dtype=np.float32))

    nc = _get_nc()
    in_maps = []
    for core in range(8):
        b, h = core // 2, core % 2
        in_maps.append({
            "x": x[b],
            "wq": W_q,
            "wk": W_k,
            "wv": np.ascontiguousarray(W_v[:, h * E:(h + 1) * E]),
        })
    res = run_bass_kernel_spmd(
        nc, in_maps, core_ids=list(range(8)),
        trace=bool(int(os.environ.get("KERNEL_TRACE", "0"))),
    )
    _cache["last_results"] = res
    attn_weights = np.stack([res.results[2 * b]["aw"] for b in range(B)])
    attn_output = np.stack([
        np.concatenate([res.results[2 * b]["ao"], res.results[2 * b + 1]["ao"]],
                       axis=1)
        for b in range(B)
    ])
    return attn_output, attn_weights
